# revision 26
# baseline (speedup 1.0000x reference)
"""Trainium2 Bass kernel for nn_EncoderNoResidual (GNN message passing).

Row-parallel across 8 NeuronCores: core c owns nodes [c*512, (c+1)*512).
Activations kept in transposed [feature, node] layout on-chip so the PE
contracts over the partition dim.  adj / graph_neigh row-blocks are
cast to bf16 in-flight (SWDGE DMA cast; 0/1 values are exact in bf16),
transposed on-chip once (batched PE transposes) and kept resident in
SBUF.  h = adj @ (z @ W2) is computed as (adj @ z) @ W2 (associativity:
3.7 GFLOP instead of 100).  Softmax skips max-subtraction (logits are
O(10)); non-edges are masked by multiplying exp(e) with the 0/1
adjacency, matching the reference's -9e15 mask.  PE-heavy matmuls run
in bf16 (fp32 matmul is 2-pass on TRN2); accumulation stays fp32.

Cross-core traffic: one merged AllGather of [2, 66, 512] (Wh|1|dst for
both attention branches) + two [64, 512] AllGathers (z, z_a).
Outputs are per-core row blocks assembled on host.
"""

import sys

sys.path.insert(0, "/opt/trn_rl_repo")

import numpy as np

import concourse.bass as bass
import concourse.tile as tile
from concourse import mybir
from concourse.bass_utils import run_bass_kernel_spmd
from concourse.masks import make_identity
import bass_rust

FP = mybir.dt.float32
BF = mybir.dt.float16  # 16-bit compute dtype (fp16: 11-bit mantissa)
AF = mybir.ActivationFunctionType
ALU = mybir.AluOpType

N = 4096
F_IN = 3000
F_OUT = 64
NCORES = 8
NB = N // NCORES          # 512 nodes per core
NT = NB // 128            # 4 node-tiles per core
JT = N // 128             # 32 j-tiles globally
KT = (F_IN + 127) // 128  # 24 k-tiles over F_IN (last = 56)
LAST_K = F_IN - 128 * (KT - 1)


def _split_waits(nc, max_waits=1):
    """This container's walrus accepts only ONE sync wait per instruction.
    Move excess waits onto NoOps inserted just before the offending
    instruction on the same engine (streams are in-order, so this is
    semantically identical)."""
    n = 0
    for fn in nc.m.functions:
        for bb in fn.blocks:
            out = []
            changed = False
            for ins in list(bb.instructions):
                si = ins.sync_info
                waits = list(si.on_wait) if si is not None else []
                if len(waits) > max_waits:
                    changed = True
                    n += 1
                    head, rest = waits[:-max_waits], waits[-max_waits:]
                    for i in range(0, len(head), max_waits):
                        out.append(
                            mybir.InstNoOp(
                                name=f"{ins.name}_ws{i}",
                                engine=ins.engine,
                                ins=[],
                                outs=[],
                                sync_info=bass_rust.SyncInfo(
                                    on_wait=head[i : i + max_waits], on_update=[]
                                ),
                            )
                        )
                    ins.sync_info = bass_rust.SyncInfo(
                        on_wait=rest, on_update=list(si.on_update)
                    )
                out.append(ins)
            if changed:
                bb.instructions = out
    return n


def _build(nc):
    io = {}
    def din(name, shape):
        io[name] = nc.dram_tensor(name, shape, FP, kind="ExternalInput")
    def dout(name, shape):
        io[name] = nc.dram_tensor(name, shape, FP, kind="ExternalOutput")

    din("feat", [NB, F_IN]); din("feat_a", [NB, F_IN])
    din("adj", [NB, N]); din("graph_neigh", [NB, N])
    din("weight1", [F_IN, F_OUT]); din("weight2", [F_OUT, F_IN])
    din("att_W", [F_OUT, F_OUT]); din("a_src", [F_OUT, 1]); din("a_dst", [F_OUT, 1])
    din("mlp_W1", [64, 256]); din("mlp_b1", [256])
    din("mlp_W2", [256, 128]); din("mlp_b2", [128])
    din("mlp_W3", [128, 64]); din("mlp_b3", [64])
    din("dmlp_W1", [64, 128]); din("dmlp_b1", [128])
    din("dmlp_W2", [128, 64]); din("dmlp_b2", [64])
    din("bil_W", [64, 64]); din("bil_b", [1])
    dout("hiden_emb_T", [64, NB]); dout("h", [NB, F_IN])
    dout("ret_T", [2, NB]); dout("ret_a_T", [2, NB])

    with tile.TileContext(nc) as tc:
        _graph(nc, tc, io)
    return nc


def _graph(nc, tc, io):
    from contextlib import ExitStack

    RG = [list(range(NCORES))]

    ctx = ExitStack()
    with ctx:
        const = ctx.enter_context(tc.tile_pool(name="const", bufs=1))
        bigA = ctx.enter_context(tc.tile_pool(name="bigA", bufs=1))
        bigG = ctx.enter_context(tc.tile_pool(name="bigG", bufs=1))
        natp = ctx.enter_context(tc.tile_pool(name="natp", bufs=3))
        fTp = ctx.enter_context(tc.tile_pool(name="fTp", bufs=1))
        whoTp = ctx.enter_context(tc.tile_pool(name="whoTp", bufs=1))
        embp = ctx.enter_context(tc.tile_pool(name="embp", bufs=2))
        zfp = ctx.enter_context(tc.tile_pool(name="zfp", bufs=1))
        gchunk = ctx.enter_context(tc.tile_pool(name="gchunk", bufs=2))
        ebuf = ctx.enter_context(tc.tile_pool(name="ebuf", bufs=3))
        srcp = ctx.enter_context(tc.tile_pool(name="srcp", bufs=2))
        hstp = ctx.enter_context(tc.tile_pool(name="hstp", bufs=2))
        longp = ctx.enter_context(tc.tile_pool(name="longp", bufs=1))
        scr = ctx.enter_context(tc.tile_pool(name="scr", bufs=5))
        x1p = ctx.enter_context(tc.tile_pool(name="x1p", bufs=1))
        rows = ctx.enter_context(tc.tile_pool(name="rows", bufs=3))
        dram = ctx.enter_context(tc.tile_pool(name="dram", bufs=1, space="DRAM"))
        psT = ctx.enter_context(tc.tile_pool(name="psT", bufs=3, space="PSUM"))
        psMM = ctx.enter_context(tc.tile_pool(name="psMM", bufs=2, space="PSUM"))
        psA = ctx.enter_context(tc.tile_pool(name="psA", bufs=2, space="PSUM"))
        psR = ctx.enter_context(tc.tile_pool(name="psR", bufs=1, space="PSUM"))

        # ---------------- constants / weights ----------------
        identb = const.tile([128, 128], BF, tag="identb")
        make_identity(nc, identb)
        identf = const.tile([128, 128], FP, tag="identf")
        make_identity(nc, identf)
        ones1 = const.tile([1, 128], FP, tag="ones1")
        nc.vector.memset(ones1, 1.0)
        ones64 = const.tile([64, 1], FP, tag="ones64")
        nc.vector.memset(ones64, 1.0)
        onesR = const.tile([1, NB], FP, tag="onesR")
        nc.vector.memset(onesR, 1.0)
        onesRh = const.tile([1, NB], BF, tag="onesRh")
        nc.vector.memset(onesRh, 1.0)
        negC = const.tile([128, 1], FP, tag="negC")
        nc.vector.memset(negC, -8.0)  # exp(e-8): keeps A in fp16 range; cancels in softmax norm

        # bf16 weights via SWDGE cast DMA
        w1_sb = const.tile([128, KT, F_OUT], BF, tag="w1")
        nc.gpsimd.dma_start(
            w1_sb[:, : KT - 1, :],
            io["weight1"][: 128 * (KT - 1)].rearrange("(t p) f -> p t f", p=128),
        )
        nc.gpsimd.dma_start(w1_sb[:LAST_K, KT - 1, :], io["weight1"][128 * (KT - 1):])
        w2_sb = const.tile([64, F_IN], BF, tag="w2")
        nc.gpsimd.dma_start(w2_sb, io["weight2"][:])

        attw_sb = const.tile([64, 64], FP, tag="attw")
        nc.sync.dma_start(attw_sb, io["att_W"][:])
        asrc_sb = const.tile([64, 1], FP, tag="asrc")
        nc.sync.dma_start(asrc_sb, io["a_src"][:])
        adst_sb = const.tile([64, 1], FP, tag="adst")
        nc.sync.dma_start(adst_sb, io["a_dst"][:])
        mw1_sb = const.tile([64, 256], FP, tag="mw1")
        nc.sync.dma_start(mw1_sb, io["mlp_W1"][:])
        mw2_sb = const.tile([128, 2, 128], FP, tag="mw2")
        nc.sync.dma_start(mw2_sb, io["mlp_W2"][:].rearrange("(t p) f -> p t f", p=128))
        mw3_sb = const.tile([128, 64], FP, tag="mw3")
        nc.sync.dma_start(mw3_sb, io["mlp_W3"][:])
        dw1_sb = const.tile([64, 128], FP, tag="dw1")
        nc.sync.dma_start(dw1_sb, io["dmlp_W1"][:])
        dw2_sb = const.tile([128, 64], FP, tag="dw2")
        nc.sync.dma_start(dw2_sb, io["dmlp_W2"][:])
        bilw_sb = const.tile([64, 64], FP, tag="bilw")
        nc.sync.dma_start(bilw_sb, io["bil_W"][:])

        b1_sb = const.tile([128, 2], FP, tag="b1")
        nc.sync.dma_start(b1_sb, io["mlp_b1"][:].rearrange("(t p) -> p t", p=128))
        b2_sb = const.tile([128, 1], FP, tag="b2")
        nc.sync.dma_start(b2_sb, io["mlp_b2"][:].rearrange("(p x) -> p x", x=1))
        b3_sb = const.tile([64, 1], FP, tag="b3")
        nc.sync.dma_start(b3_sb, io["mlp_b3"][:].rearrange("(p x) -> p x", x=1))
        db1_sb = const.tile([128, 1], FP, tag="db1")
        nc.sync.dma_start(db1_sb, io["dmlp_b1"][:].rearrange("(p x) -> p x", x=1))
        db2_sb = const.tile([64, 1], FP, tag="db2")
        nc.sync.dma_start(db2_sb, io["dmlp_b2"][:].rearrange("(p x) -> p x", x=1))
        bilb_sb = const.tile([1, 1], FP, tag="bilb")
        nc.sync.dma_start(bilb_sb, io["bil_b"][:].rearrange("(p x) -> p x", x=1))

        pt0 = psT.tile([128, 1024], BF, tag="pt")
        ptf0 = pt0.bitcast(FP)  # reuse the bank for one fp32 transpose
        nc.tensor.transpose(ptf0[:64, :64], bilw_sb, identf[:64, :64])
        bilwT_sb = const.tile([64, 64], FP, tag="bilwT")
        nc.vector.tensor_copy(bilwT_sb, ptf0[:64, :64])

        def psum_copy(dst, src):
            nc.vector.tensor_copy(dst, src)

        def act_recip(out, in_, scale=-1.0, clamp=None):
            # 1/x (scale=-1) or 1/sqrt(x) (scale=-0.5) via exp(scale*ln(x));
            # Ln+Exp live in the same ACT table set as Prelu/Relu.
            t = rows.tile([1, NB], FP, tag="rows", name="lnf")
            if clamp is not None:
                nc.vector.tensor_scalar_max(t, in_, clamp)
                nc.scalar.activation(t, t, AF.Ln)
            else:
                nc.scalar.activation(t, in_, AF.Ln)
            nc.scalar.activation(out, t, AF.Exp, scale=scale)

        # ------------- branch pre: feat -> h1T, who rows in DRAM -------
        def feat_group(fT, fname, it, q):
            c0 = q * 1024
            cw = min(1024, F_IN - c0)
            nk = (cw + 127) // 128
            nat = natp.tile([128, 1024], BF, tag="nat", name="fnat")
            nc.gpsimd.dma_start(
                nat[:, :cw],
                io[fname][it * 128 : (it + 1) * 128, c0 : c0 + cw],
            )
            ptt = psT.tile([128, 1024], BF, tag="pt", name="fptt")
            for kl in range(nk):
                kt = q * 8 + kl
                kc = LAST_K if kt == KT - 1 else 128
                nc.tensor.transpose(
                    ptt[:kc, kl * 128 : (kl + 1) * 128],
                    nat[:, kl * 128 : kl * 128 + kc],
                    identb,
                )
            psum_copy(
                fT[:, q * 8 : q * 8 + nk, it * 128 : (it + 1) * 128],
                ptt.rearrange("p (k n) -> p k n", k=8)[:, :nk, :],
            )

        def branch_tail(fT, who_d, srcB):
            h1T = scr.tile([64, NB], FP, tag="sc")
            pm = psMM.tile([64, NB], FP, tag="mm")
            for kt in range(KT):
                kc = LAST_K if kt == KT - 1 else 128
                nc.tensor.matmul(
                    pm, w1_sb[:kc, kt, :], fT[:kc, kt, :],
                    start=(kt == 0), stop=(kt == KT - 1),
                )
            nc.vector.tensor_copy(h1T, pm)

            # who rows: 0:64 Wh^T = att_W.T @ h1^T ; 64 ones ; 65 dst^T
            whT = scr.tile([64, NB], FP, tag="sc")
            whTh = scr.tile([64, NB], BF, tag="sch", bufs=2)
            pw = psMM.tile([64, NB], FP, tag="mm")
            nc.tensor.matmul(pw, attw_sb, h1T, start=True, stop=True)
            nc.vector.tensor_copy(whT, pw)
            nc.scalar.copy(whTh, pw)
            pd = psR.tile([1, NB], FP, tag="row")
            nc.tensor.matmul(pd, adst_sb, whT, start=True, stop=True)
            dstR = rows.tile([1, NB], BF, tag="rows")
            nc.vector.tensor_copy(dstR, pd)

            ps = psR.tile([1, NB], FP, tag="row")
            nc.tensor.matmul(ps, asrc_sb, whT, start=True, stop=True)
            src_sb = rows.tile([1, NB], FP, tag="rows")
            nc.vector.tensor_copy(src_sb, ps)
            pb = psMM.tile([128, NB], FP, tag="mm")
            nc.tensor.matmul(pb, ones1, src_sb, start=True, stop=True)
            nc.vector.tensor_copy(srcB, pb)

            nc.sync.dma_start(who_d[0:64, :], whTh)
            nc.sync.dma_start(who_d[64:65, :], onesRh)
            nc.sync.dma_start(who_d[65:66, :], dstR)

        # ------------- branch post: whoT tiles (bf16 lhsT + fp32 dst) --
        def branch_post(whoF_d):
            whoTb = whoTp.tile([128, JT, 65], BF, tag="whoTb")
            dstc = whoTp.tile([128, JT, 1], FP, tag="dstc")
            for c8 in range(NCORES):
                wf = gchunk.tile([66, NB], BF, tag="gch")
                nc.sync.dma_start(wf, whoF_d[c8])
                ptt = psT.tile([128, 1024], BF, tag="pt")
                pth = ptt.rearrange("p (k n) -> p k n", k=8)
                for nt8 in range(NT):
                    nc.tensor.transpose(
                        pth[:, nt8, 0:66],
                        wf[:, nt8 * 128 : (nt8 + 1) * 128],
                        identb[:66, :66],
                    )
                jt0 = c8 * NT
                psum_copy(whoTb[:, jt0 : jt0 + 4, :], pth[:, 0:NT, 0:65])
                psum_copy(dstc[:, jt0 : jt0 + 4, :], pth[:, 0:NT, 65:66])
            return whoTb, dstc

        # ---------------- attention ----------------
        def attention(srcB, whoTb, dstc, adjT, fillers=(), start_jt=0):
            fillers = list(fillers)
            pa = psA.tile([65, NB], FP, tag="acc")
            for jt in range(JT):
                et = ebuf.tile([128, NB], FP, tag="et")
                nc.scalar.activation(
                    et, srcB, AF.Prelu,
                    bias=dstc[:, jt, :], scale=1.0, alpha=0.2,
                )
                at = ebuf.tile([128, NB], BF, tag="at")
                nc.scalar.activation(at, et, AF.Exp, bias=negC[:, 0:1])
                nc.vector.tensor_mul(at, at, adjT[:, jt, :])
                nc.tensor.matmul(
                    pa, whoTb[:, jt, :], at,
                    start=(jt == 0), stop=(jt == JT - 1),
                )
                if jt >= start_jt and fillers:
                    fillers.pop(0)()
            while fillers:
                fillers.pop(0)()
            ao = scr.tile([65, NB], FP, tag="sc")
            nc.vector.tensor_copy(ao, pa)
            rs = rows.tile([1, NB], FP, tag="rows")
            act_recip(rs, ao[64:65, :])
            pb = psMM.tile([64, NB], FP, tag="mm")
            nc.tensor.matmul(pb, ones1[:, :64], rs, start=True, stop=True)
            hp = scr.tile([64, NB], FP, tag="sc")
            nc.vector.tensor_mul(hp, ao[0:64, :], pb)
            return hp

        # ---------------- mlp ----------------
        def mlp(hp, ztag):
            x1 = x1p.tile([128, 2, NB], FP, tag="x1")
            for chk in range(2):
                pm = psMM.tile([128, NB], FP, tag="mm")
                nc.tensor.matmul(
                    pm, mw1_sb[:, chk * 128 : (chk + 1) * 128], hp,
                    start=True, stop=True,
                )
                nc.scalar.activation(
                    x1[:, chk, :], pm, AF.Relu, bias=b1_sb[:, chk : chk + 1]
                )
            pm2 = psMM.tile([128, NB], FP, tag="mm")
            for kt in range(2):
                nc.tensor.matmul(
                    pm2, mw2_sb[:, kt, :], x1[:, kt, :],
                    start=(kt == 0), stop=(kt == 1),
                )
            x2 = scr.tile([128, NB], FP, tag="sc")
            nc.scalar.activation(x2, pm2, AF.Relu, bias=b2_sb[:, 0:1])
            pm3 = psMM.tile([64, NB], FP, tag="mm")
            nc.tensor.matmul(pm3, mw3_sb, x2, start=True, stop=True)
            zT = longp.tile([64, NB], FP, tag=ztag)
            nc.scalar.activation(zT, pm3, AF.Identity, bias=b3_sb[:, 0:1])
            return zT

        # big transposed 0/1 matrices (fp16, exact)
        def bigT_group(dst, src_name, it, q):
            nat = natp.tile([128, 1024], BF, tag="nat", name="nat")
            nc.gpsimd.dma_start(
                nat,
                io[src_name][it * 128 : (it + 1) * 128,
                             q * 1024 : (q + 1) * 1024],
            )
            ptt = psT.tile([128, 1024], BF, tag="pt", name="ptt")
            for jl in range(8):
                nc.tensor.transpose(
                    ptt[:, jl * 128 : (jl + 1) * 128],
                    nat[:, jl * 128 : (jl + 1) * 128],
                    identb,
                )
            psum_copy(
                dst[:, q * 8 : (q + 1) * 8, it * 128 : (it + 1) * 128],
                ptt.rearrange("p (k n) -> p k n", k=8),
            )

        def build_bigT(dst, src_name, groups=None):
            for (it, q) in (groups or [(i, q) for i in range(NT) for q in range(4)]):
                bigT_group(dst, src_name, it, q)

        def dmlp(x, tagp):
            pm = psMM.tile([128, NB], FP, tag="mm", name="dpm")
            nc.tensor.matmul(pm, dw1_sb, x, start=True, stop=True)
            t = scr.tile([128, NB], FP, tag="sc", name="dt")
            nc.scalar.activation(t, pm, AF.Relu, bias=db1_sb[:, 0:1])
            pm2 = psMM.tile([64, NB], FP, tag="mm", name="dpm2")
            nc.tensor.matmul(pm2, dw2_sb, t, start=True, stop=True)
            c = longp.tile([64, NB], FP, tag=tagp, name="dmc")
            nc.scalar.activation(c, pm2, AF.Identity, bias=db2_sb[:, 0:1])
            return c

        # ================= schedule =================
        # tiny dependency-free collective up front absorbs comm setup cost
        warm_d = dram.tile([1, 64], FP, tag="warm_d")
        warmrow = const.tile([1, 64], FP, tag="warmrow", name="warmrow")
        nc.vector.memset(warmrow, 0.0)
        nc.sync.dma_start(warm_d[:], warmrow)
        warmF_d = dram.tile([NCORES, 1, 64], FP, tag="warmF_d")
        nc.gpsimd.collective_compute(
            "AllGather", ALU.bypass, replica_groups=RG,
            ins=[warm_d[:].opt()], outs=[warmF_d[:].opt()],
        )

        who1_d = dram.tile([66, NB], BF, tag="who1_d")
        who2_d = dram.tile([66, NB], BF, tag="who2_d")
        whoF1_d = dram.tile([NCORES, 66, NB], BF, tag="whoF1_d")
        whoF2_d = dram.tile([NCORES, 66, NB], BF, tag="whoF2_d")

        fT1 = fTp.tile([128, KT, NB], BF, tag="fT", name="fT1")
        srcB1 = srcp.tile([128, NB], FP, tag="srcB", name="srcB1")
        for it in range(NT):
            for q in range(3):
                feat_group(fT1, "feat", it, q)
        branch_tail(fT1, who1_d, srcB1)
        nc.gpsimd.collective_compute(
            "AllGather", ALU.bypass, replica_groups=RG,
            ins=[who1_d[:].opt()], outs=[whoF1_d[:].opt()],
        )

        adjT = bigA.tile([128, JT, 512], BF, tag="adjT")
        gnT = bigG.tile([128, JT, 512], BF, tag="gnT")
        # q=0,1 of adjT up front (covers jt 0..15); rest threads through attn1
        build_bigT(adjT, "adj", [(i, q) for q in (0, 1) for i in range(NT)])

        whoTb1, dstc1 = branch_post(whoF1_d)

        # branch-2 pre-work + remaining transposed builds all run as
        # fillers inside attention1 (which is ACT-bound)
        fT2 = fTp.tile([128, KT, NB], BF, tag="fT", name="fT2")
        srcB2 = srcp.tile([128, NB], FP, tag="srcB", name="srcB2")
        b2_groups = [(it, q) for it in range(NT) for q in range(3)]
        adj_q = {q: [(i, q) for i in range(NT)] for q in (1, 2, 3)}
        gn_groups = [(i, q) for i in range(NT) for q in range(4)]

        def mk_bigT(dst, name, it, q):
            return lambda: bigT_group(dst, name, it, q)
        def mk_feat(it, q):
            return lambda: feat_group(fT2, "feat_a", it, q)

        fillers = (
            [mk_feat(it, q) for (it, q) in b2_groups]              # jt 0-11
            + [mk_bigT(adjT, "adj", it, q) for (it, q) in adj_q[2]]  # jt 12-15
            + [lambda: branch_tail(fT2, who2_d, srcB2)]             # jt 16
            + [lambda: nc.gpsimd.collective_compute(
                "AllGather", ALU.bypass, replica_groups=RG,
                ins=[who2_d[:].opt()], outs=[whoF2_d[:].opt()])]    # jt 17
            + [mk_bigT(adjT, "adj", it, q) for (it, q) in adj_q[3]]  # jt 18-21 (needed jt 24)
            + [mk_bigT(gnT, "graph_neigh", it, q) for (it, q) in gn_groups[:10]]  # jt 22-31
        )
        hp1 = attention(srcB1, whoTb1, dstc1, adjT, fillers=fillers)
        zT = mlp(hp1, "zT")
        nc.sync.dma_start(io["hiden_emb_T"][:], zT)
        embT = longp.tile([64, NB], FP, tag="embT")
        nc.scalar.activation(embT, zT, AF.Relu)
        dm_e = dmlp(embT, "dme")

        # z AllGather (launch early; overlaps attention2)
        z_d = dram.tile([64, NB], FP, tag="z_d")
        nc.sync.dma_start(z_d[:], zT)
        zF_d = dram.tile([NCORES, 64, NB], FP, tag="zF_d")
        nc.gpsimd.collective_compute(
            "AllGather", ALU.bypass, replica_groups=RG,
            ins=[z_d[:].opt()], outs=[zF_d[:].opt()],
        )

        zfull = zfp.tile([128, JT, 64], BF, tag="zfull")
        embO = embp.tile([128, JT, 65], BF, tag="embO")
        nc.vector.memset(embO[:, :, 64:65], 1.0)

        def z_group(c8):
            zc = gchunk.tile([66, NB], FP, tag="gch", name="zc")
            nc.sync.dma_start(zc[:64, :], zF_d[c8])
            ptt = psT.tile([128, 1024], BF, tag="pt", name="zpt")
            ptf = ptt.bitcast(FP).rearrange("p (k n) -> p k n", k=4)
            for nt8 in range(NT):
                nc.tensor.transpose(
                    ptf[:, nt8, 0:64],
                    zc[:64, nt8 * 128 : (nt8 + 1) * 128],
                    identf[:64, :64],
                )
            jt0 = c8 * NT
            psum_copy(zfull[:, jt0 : jt0 + 4, :], ptf[:, :, 0:64])
            nc.vector.tensor_relu(embO[:, jt0 : jt0 + 4, 0:64], ptf[:, :, 0:64])

        whoTb2, dstc2 = branch_post(whoF2_d)
        hp2 = attention(
            srcB2, whoTb2, dstc2, adjT,
            fillers=[mk_bigT(gnT, "graph_neigh", it, q) for (it, q) in gn_groups[10:]]
            + [(lambda c8_=c8: z_group(c8_)) for c8 in range(NCORES)],
            start_jt=10,
        )
        zaT = mlp(hp2, "zaT")
        embaT = longp.tile([64, NB], FP, tag="embaT")
        nc.scalar.activation(embaT, zaT, AF.Relu)
        dm_ea = dmlp(embaT, "dmea")

        za_d = dram.tile([64, NB], FP, tag="za_d")
        nc.sync.dma_start(za_d[:], zaT)
        zaF_d = dram.tile([NCORES, 64, NB], FP, tag="zaF_d")
        nc.gpsimd.collective_compute(
            "AllGather", ALU.bypass, replica_groups=RG,
            ins=[za_d[:].opt()], outs=[zaF_d[:].opt()],
        )

        # az = adj @ z (transposed), then h = az @ W2 (natural rows)
        paz = psA.tile([64, NB], FP, tag="acc")
        for jt in range(JT):
            nc.tensor.matmul(
                paz, zfull[:, jt, :], adjT[:, jt, :],
                start=(jt == 0), stop=(jt == JT - 1),
            )
        azT = longp.tile([64, NB], BF, tag="azT")
        nc.vector.tensor_copy(azT, paz)

        MCW = 500
        for it in range(NT):
            for mh in range(3):
                hs = hstp.tile([128, 2 * MCW], FP, tag="hst")
                for ml in range(2):
                    mc = mh * 2 + ml
                    pm = psMM.tile([128, MCW], FP, tag="mm")
                    nc.tensor.matmul(
                        pm,
                        azT[:, it * 128 : (it + 1) * 128],
                        w2_sb[:, mc * MCW : (mc + 1) * MCW],
                        start=True, stop=True,
                    )
                    psum_copy(hs[:, ml * MCW : (ml + 1) * MCW], pm)
                nc.sync.dma_start(
                    io["h"][it * 128 : (it + 1) * 128,
                            mh * 2 * MCW : (mh + 1) * 2 * MCW],
                    hs,
                )

        # readout pass 1 over resident gn^T tiles (N=512 matmuls)
        pro1 = psA.tile([65, NB], FP, tag="acc")
        for jt in range(JT):
            nc.tensor.matmul(
                pro1, embO[:, jt, :], gnT[:, jt, :],
                start=(jt == 0), stop=(jt == JT - 1),
            )

        # shared: 1/rowsum(graph_neigh) (row 64 of either pro)
        rsg = rows.tile([1, NB], FP, tag="rows")
        rog = scr.tile([65, NB], FP, tag="sc", name="rog")
        nc.vector.tensor_copy(rog, pro1)
        act_recip(rsg, rog[64:65, :])
        pbg = psMM.tile([64, NB], FP, tag="mm")
        nc.tensor.matmul(pbg, ones1[:, :64], rsg, start=True, stop=True)
        rsgb = longp.tile([64, NB], FP, tag="rsgb")
        nc.vector.tensor_copy(rsgb, pbg)

        def readout_finish(ro64, gtag):
            gpre = scr.tile([64, NB], FP, tag="sc", name="gpre")
            nc.vector.tensor_mul(gpre, ro64, rsgb)
            sq = scr.tile([64, NB], FP, tag="sc", name="sq")
            nc.scalar.activation(sq, gpre, AF.Square)
            pn = psR.tile([1, NB], FP, tag="row", name="pn")
            nc.tensor.matmul(pn, ones64, sq, start=True, stop=True)
            rn = rows.tile([1, NB], FP, tag="rows", name="rn")
            act_recip(rn, pn, scale=-0.5, clamp=1e-24)
            pb2 = psMM.tile([64, NB], FP, tag="mm", name="pb2")
            nc.tensor.matmul(pb2, ones1[:, :64], rn, start=True, stop=True)
            gg = scr.tile([64, NB], FP, tag="sc", name="gg")
            nc.vector.tensor_mul(gg, gpre, pb2)
            g = longp.tile([64, NB], FP, tag=gtag, name="g")
            nc.scalar.activation(g, gg, AF.Sigmoid)
            return g

        def bilinear(x, y, out_ap):
            pu = psMM.tile([64, NB], FP, tag="mm", name="pu")
            nc.tensor.matmul(pu, bilwT_sb, y, start=True, stop=True)
            p = scr.tile([64, NB], FP, tag="sc", name="bp")
            nc.vector.tensor_mul(p, x, pu)
            pr = psR.tile([1, NB], FP, tag="row", name="pr")
            nc.tensor.matmul(pr, ones64, p, start=True, stop=True)
            nc.scalar.activation(out_ap, pr, AF.Identity, bias=bilb_sb[:, 0:1])

        # g1 branch finishes while the g2 branch still waits on za-AG
        g1 = readout_finish(rog[0:64, :], "g1")
        dm_g = dmlp(g1, "dmg")
        r00 = longp.tile([1, NB], FP, tag="r00")
        r01 = longp.tile([1, NB], FP, tag="r01")
        bilinear(dm_e, dm_g, r00)
        bilinear(dm_ea, dm_g, r01)
        nc.sync.dma_start(io["ret_T"][0:1, :], r00)
        nc.sync.dma_start(io["ret_T"][1:2, :], r01)

        # embaO tiles, then second readout pass
        embaO = embp.tile([128, JT, 65], BF, tag="embO")
        nc.vector.memset(embaO[:, :, 64:65], 1.0)
        for c8 in range(NCORES):
            zc = gchunk.tile([66, NB], FP, tag="gch")
            nc.sync.dma_start(zc[:64, :], zaF_d[c8])
            ptt = psT.tile([128, 1024], BF, tag="pt")
            ptf = ptt.bitcast(FP).rearrange("p (k n) -> p k n", k=4)
            for nt8 in range(NT):
                nc.tensor.transpose(
                    ptf[:, nt8, 0:64],
                    zc[:64, nt8 * 128 : (nt8 + 1) * 128],
                    identf[:64, :64],
                )
            jt0 = c8 * NT
            nc.vector.tensor_relu(embaO[:, jt0 : jt0 + 4, 0:64], ptf[:, :, 0:64])

        pro2 = psA.tile([65, NB], FP, tag="acc")
        for jt in range(JT):
            nc.tensor.matmul(
                pro2, embaO[:, jt, :], gnT[:, jt, :],
                start=(jt == 0), stop=(jt == JT - 1),
            )

        ro2s = scr.tile([65, NB], FP, tag="sc", name="ro2s")
        nc.vector.tensor_copy(ro2s, pro2)
        g2 = readout_finish(ro2s[0:64, :], "g2")
        dm_ga = dmlp(g2, "dmga")
        r10 = longp.tile([1, NB], FP, tag="r10")
        r11 = longp.tile([1, NB], FP, tag="r11")
        bilinear(dm_ea, dm_ga, r10)
        bilinear(dm_e, dm_ga, r11)
        nc.sync.dma_start(io["ret_a_T"][0:1, :], r10)
        nc.sync.dma_start(io["ret_a_T"][1:2, :], r11)


_CACHED = {}


def _get_program():
    if "nc" not in _CACHED:
        nc = bass.Bass(num_devices=NCORES)
        _build(nc)
        _split_waits(nc)
        _CACHED["nc"] = nc
    return _CACHED["nc"]


def run(inputs, **kwargs):
    nc = _get_program()
    w_names = [
        "weight1", "weight2", "att_W", "a_src", "a_dst",
        "mlp_W1", "mlp_b1", "mlp_W2", "mlp_b2", "mlp_W3", "mlp_b3",
        "dmlp_W1", "dmlp_b1", "dmlp_W2", "dmlp_b2", "bil_W", "bil_b",
    ]
    ws = {k: np.ascontiguousarray(np.asarray(inputs[k], dtype=np.float32))
          for k in w_names}
    in_maps = []
    for c in range(NCORES):
        sl = slice(c * NB, (c + 1) * NB)
        m = dict(ws)
        for k in ("feat", "feat_a", "adj", "graph_neigh"):
            m[k] = np.ascontiguousarray(
                np.asarray(inputs[k], dtype=np.float32)[sl])
        in_maps.append(m)

    res = run_bass_kernel_spmd(nc, in_maps, core_ids=list(range(NCORES)), **kwargs)
    outs = res.results
    hiden_emb = np.concatenate([o["hiden_emb_T"].T for o in outs], axis=0)
    h = np.concatenate([o["h"] for o in outs], axis=0)
    ret = np.concatenate([o["ret_T"].T for o in outs], axis=0)
    ret_a = np.concatenate([o["ret_a_T"].T for o in outs], axis=0)
    return (hiden_emb, h, ret, ret_a), res


def kernel(**inputs):
    out, _ = run(inputs)
    return out


# revision 27
# speedup vs baseline: 1.0463x; 1.0463x over previous
"""Trainium2 Bass kernel for nn_EncoderNoResidual (GNN message passing).

Row-parallel across 8 NeuronCores: core c owns nodes [c*512, (c+1)*512).
Activations kept in transposed [feature, node] layout on-chip so the PE
contracts over the partition dim.  adj / graph_neigh row-blocks are
cast to bf16 in-flight (SWDGE DMA cast; 0/1 values are exact in bf16),
transposed on-chip once (batched PE transposes) and kept resident in
SBUF.  h = adj @ (z @ W2) is computed as (adj @ z) @ W2 (associativity:
3.7 GFLOP instead of 100).  Softmax skips max-subtraction (logits are
O(10)); non-edges are masked by multiplying exp(e) with the 0/1
adjacency, matching the reference's -9e15 mask.  PE-heavy matmuls run
in bf16 (fp32 matmul is 2-pass on TRN2); accumulation stays fp32.

Cross-core traffic: one merged AllGather of [2, 66, 512] (Wh|1|dst for
both attention branches) + two [64, 512] AllGathers (z, z_a).
Outputs are per-core row blocks assembled on host.
"""

import sys

sys.path.insert(0, "/opt/trn_rl_repo")

import numpy as np

import concourse.bass as bass
import concourse.tile as tile
from concourse import mybir
from concourse.bass_utils import run_bass_kernel_spmd
from concourse.masks import make_identity
import bass_rust

FP = mybir.dt.float32
BF = mybir.dt.float16  # 16-bit compute dtype (fp16: 11-bit mantissa)
AF = mybir.ActivationFunctionType
ALU = mybir.AluOpType

N = 4096
F_IN = 3000
F_OUT = 64
NCORES = 8
NB = N // NCORES          # 512 nodes per core
NT = NB // 128            # 4 node-tiles per core
JT = N // 128             # 32 j-tiles globally
KT = (F_IN + 127) // 128  # 24 k-tiles over F_IN (last = 56)
LAST_K = F_IN - 128 * (KT - 1)


def _split_waits(nc, max_waits=1):
    """This container's walrus accepts only ONE sync wait per instruction.
    Move excess waits onto NoOps inserted just before the offending
    instruction on the same engine (streams are in-order, so this is
    semantically identical)."""
    n = 0
    for fn in nc.m.functions:
        for bb in fn.blocks:
            out = []
            changed = False
            for ins in list(bb.instructions):
                si = ins.sync_info
                waits = list(si.on_wait) if si is not None else []
                if len(waits) > max_waits:
                    changed = True
                    n += 1
                    head, rest = waits[:-max_waits], waits[-max_waits:]
                    for i in range(0, len(head), max_waits):
                        out.append(
                            mybir.InstNoOp(
                                name=f"{ins.name}_ws{i}",
                                engine=ins.engine,
                                ins=[],
                                outs=[],
                                sync_info=bass_rust.SyncInfo(
                                    on_wait=head[i : i + max_waits], on_update=[]
                                ),
                            )
                        )
                    ins.sync_info = bass_rust.SyncInfo(
                        on_wait=rest, on_update=list(si.on_update)
                    )
                out.append(ins)
            if changed:
                bb.instructions = out
    return n


def _build(nc):
    io = {}
    def din(name, shape):
        io[name] = nc.dram_tensor(name, shape, FP, kind="ExternalInput")
    def dout(name, shape):
        io[name] = nc.dram_tensor(name, shape, FP, kind="ExternalOutput")

    din("feat", [NB, F_IN]); din("feat_a", [NB, F_IN])
    din("adj", [NB, N]); din("graph_neigh", [NB, N])
    din("weight1", [F_IN, F_OUT]); din("weight2", [F_OUT, F_IN])
    din("att_W", [F_OUT, F_OUT]); din("a_src", [F_OUT, 1]); din("a_dst", [F_OUT, 1])
    din("mlp_W1", [64, 256]); din("mlp_b1", [256])
    din("mlp_W2", [256, 128]); din("mlp_b2", [128])
    din("mlp_W3", [128, 64]); din("mlp_b3", [64])
    din("dmlp_W1", [64, 128]); din("dmlp_b1", [128])
    din("dmlp_W2", [128, 64]); din("dmlp_b2", [64])
    din("bil_W", [64, 64]); din("bil_b", [1])
    dout("hiden_emb_T", [64, NB]); dout("h", [NB, F_IN])
    dout("ret_T", [2, NB]); dout("ret_a_T", [2, NB])

    with tile.TileContext(nc) as tc:
        _graph(nc, tc, io)
    return nc


def _graph(nc, tc, io):
    from contextlib import ExitStack

    RG = [list(range(NCORES))]

    ctx = ExitStack()
    with ctx:
        const = ctx.enter_context(tc.tile_pool(name="const", bufs=1))
        bigA = ctx.enter_context(tc.tile_pool(name="bigA", bufs=1))
        bigG = ctx.enter_context(tc.tile_pool(name="bigG", bufs=1))
        natp = ctx.enter_context(tc.tile_pool(name="natp", bufs=3))
        fTp = ctx.enter_context(tc.tile_pool(name="fTp", bufs=1))
        whoTp = ctx.enter_context(tc.tile_pool(name="whoTp", bufs=1))
        embp = ctx.enter_context(tc.tile_pool(name="embp", bufs=2))
        zfp = ctx.enter_context(tc.tile_pool(name="zfp", bufs=1))
        gchunk = ctx.enter_context(tc.tile_pool(name="gchunk", bufs=2))
        ebuf = ctx.enter_context(tc.tile_pool(name="ebuf", bufs=3))
        srcp = ctx.enter_context(tc.tile_pool(name="srcp", bufs=2))
        hstp = ctx.enter_context(tc.tile_pool(name="hstp", bufs=2))
        longp = ctx.enter_context(tc.tile_pool(name="longp", bufs=1))
        scr = ctx.enter_context(tc.tile_pool(name="scr", bufs=5))
        x1p = ctx.enter_context(tc.tile_pool(name="x1p", bufs=1))
        rows = ctx.enter_context(tc.tile_pool(name="rows", bufs=3))
        dram = ctx.enter_context(tc.tile_pool(name="dram", bufs=1, space="DRAM"))
        psT = ctx.enter_context(tc.tile_pool(name="psT", bufs=3, space="PSUM"))
        psMM = ctx.enter_context(tc.tile_pool(name="psMM", bufs=2, space="PSUM"))
        psA = ctx.enter_context(tc.tile_pool(name="psA", bufs=2, space="PSUM"))
        psR = ctx.enter_context(tc.tile_pool(name="psR", bufs=1, space="PSUM"))

        # ---------------- constants / weights ----------------
        identb = const.tile([128, 128], BF, tag="identb")
        make_identity(nc, identb)
        identf = const.tile([128, 128], FP, tag="identf")
        make_identity(nc, identf)
        ones1 = const.tile([1, 128], FP, tag="ones1")
        nc.vector.memset(ones1, 1.0)
        ones64 = const.tile([64, 1], FP, tag="ones64")
        nc.vector.memset(ones64, 1.0)
        onesR = const.tile([1, NB], FP, tag="onesR")
        nc.vector.memset(onesR, 1.0)
        onesRh = const.tile([1, NB], BF, tag="onesRh")
        nc.vector.memset(onesRh, 1.0)
        negC = const.tile([128, 1], FP, tag="negC")
        nc.vector.memset(negC, -8.0)  # exp(e-8): keeps A in fp16 range; cancels in softmax norm

        # bf16 weights via SWDGE cast DMA
        w1_sb = const.tile([128, KT, F_OUT], BF, tag="w1")
        nc.gpsimd.dma_start(
            w1_sb[:, : KT - 1, :],
            io["weight1"][: 128 * (KT - 1)].rearrange("(t p) f -> p t f", p=128),
        )
        nc.gpsimd.dma_start(w1_sb[:LAST_K, KT - 1, :], io["weight1"][128 * (KT - 1):])
        w2_sb = const.tile([64, F_IN], BF, tag="w2")
        nc.gpsimd.dma_start(w2_sb, io["weight2"][:])

        attw_sb = const.tile([64, 64], FP, tag="attw")
        nc.sync.dma_start(attw_sb, io["att_W"][:])
        asrc_sb = const.tile([64, 1], FP, tag="asrc")
        nc.sync.dma_start(asrc_sb, io["a_src"][:])
        adst_sb = const.tile([64, 1], FP, tag="adst")
        nc.sync.dma_start(adst_sb, io["a_dst"][:])
        mw1_sb = const.tile([64, 256], FP, tag="mw1")
        nc.sync.dma_start(mw1_sb, io["mlp_W1"][:])
        mw2_sb = const.tile([128, 2, 128], FP, tag="mw2")
        nc.sync.dma_start(mw2_sb, io["mlp_W2"][:].rearrange("(t p) f -> p t f", p=128))
        mw3_sb = const.tile([128, 64], FP, tag="mw3")
        nc.sync.dma_start(mw3_sb, io["mlp_W3"][:])
        dw1_sb = const.tile([64, 128], FP, tag="dw1")
        nc.sync.dma_start(dw1_sb, io["dmlp_W1"][:])
        dw2_sb = const.tile([128, 64], FP, tag="dw2")
        nc.sync.dma_start(dw2_sb, io["dmlp_W2"][:])
        bilw_sb = const.tile([64, 64], FP, tag="bilw")
        nc.sync.dma_start(bilw_sb, io["bil_W"][:])

        b1_sb = const.tile([128, 2], FP, tag="b1")
        nc.sync.dma_start(b1_sb, io["mlp_b1"][:].rearrange("(t p) -> p t", p=128))
        b2_sb = const.tile([128, 1], FP, tag="b2")
        nc.sync.dma_start(b2_sb, io["mlp_b2"][:].rearrange("(p x) -> p x", x=1))
        b3_sb = const.tile([64, 1], FP, tag="b3")
        nc.sync.dma_start(b3_sb, io["mlp_b3"][:].rearrange("(p x) -> p x", x=1))
        db1_sb = const.tile([128, 1], FP, tag="db1")
        nc.sync.dma_start(db1_sb, io["dmlp_b1"][:].rearrange("(p x) -> p x", x=1))
        db2_sb = const.tile([64, 1], FP, tag="db2")
        nc.sync.dma_start(db2_sb, io["dmlp_b2"][:].rearrange("(p x) -> p x", x=1))
        bilb_sb = const.tile([1, 1], FP, tag="bilb")
        nc.sync.dma_start(bilb_sb, io["bil_b"][:].rearrange("(p x) -> p x", x=1))

        pt0 = psT.tile([128, 1024], BF, tag="pt")
        ptf0 = pt0.bitcast(FP)  # reuse the bank for one fp32 transpose
        nc.tensor.transpose(ptf0[:64, :64], bilw_sb, identf[:64, :64])
        bilwT_sb = const.tile([64, 64], FP, tag="bilwT")
        nc.vector.tensor_copy(bilwT_sb, ptf0[:64, :64])

        def psum_copy(dst, src):
            nc.vector.tensor_copy(dst, src)

        def act_recip(out, in_, scale=-1.0, clamp=None):
            # 1/x (scale=-1) or 1/sqrt(x) (scale=-0.5) via exp(scale*ln(x));
            # Ln+Exp live in the same ACT table set as Prelu/Relu.
            t = rows.tile([1, NB], FP, tag="rows", name="lnf")
            if clamp is not None:
                nc.vector.tensor_scalar_max(t, in_, clamp)
                nc.scalar.activation(t, t, AF.Ln)
            else:
                nc.scalar.activation(t, in_, AF.Ln)
            nc.scalar.activation(out, t, AF.Exp, scale=scale)

        # ------------- branch pre: feat -> h1T, who rows in DRAM -------
        def feat_group(fT, fname, it, q):
            c0 = q * 1024
            cw = min(1024, F_IN - c0)
            nk = (cw + 127) // 128
            nat = natp.tile([128, 1024], BF, tag="nat", name="fnat")
            nc.gpsimd.dma_start(
                nat[:, :cw],
                io[fname][it * 128 : (it + 1) * 128, c0 : c0 + cw],
            )
            ptt = psT.tile([128, 1024], BF, tag="pt", name="fptt")
            for kl in range(nk):
                kt = q * 8 + kl
                kc = LAST_K if kt == KT - 1 else 128
                nc.tensor.transpose(
                    ptt[:kc, kl * 128 : (kl + 1) * 128],
                    nat[:, kl * 128 : kl * 128 + kc],
                    identb,
                )
            psum_copy(
                fT[:, q * 8 : q * 8 + nk, it * 128 : (it + 1) * 128],
                ptt.rearrange("p (k n) -> p k n", k=8)[:, :nk, :],
            )

        def branch_tail(fT, who_d, srcB):
            h1T = scr.tile([64, NB], FP, tag="sc")
            pm = psMM.tile([64, NB], FP, tag="mm")
            for kt in range(KT):
                kc = LAST_K if kt == KT - 1 else 128
                nc.tensor.matmul(
                    pm, w1_sb[:kc, kt, :], fT[:kc, kt, :],
                    start=(kt == 0), stop=(kt == KT - 1),
                )
            nc.vector.tensor_copy(h1T, pm)

            # who rows: 0:64 Wh^T = att_W.T @ h1^T ; 64 ones ; 65 dst^T
            whT = scr.tile([64, NB], FP, tag="sc")
            whTh = scr.tile([64, NB], BF, tag="sch", bufs=2)
            pw = psMM.tile([64, NB], FP, tag="mm")
            nc.tensor.matmul(pw, attw_sb, h1T, start=True, stop=True)
            nc.vector.tensor_copy(whT, pw)
            nc.scalar.copy(whTh, pw)
            pd = psR.tile([1, NB], FP, tag="row")
            nc.tensor.matmul(pd, adst_sb, whT, start=True, stop=True)
            dstR = rows.tile([1, NB], BF, tag="rows")
            nc.vector.tensor_copy(dstR, pd)

            ps = psR.tile([1, NB], FP, tag="row")
            nc.tensor.matmul(ps, asrc_sb, whT, start=True, stop=True)
            src_sb = rows.tile([1, NB], FP, tag="rows")
            nc.vector.tensor_copy(src_sb, ps)
            pb = psMM.tile([128, NB], FP, tag="mm")
            nc.tensor.matmul(pb, ones1, src_sb, start=True, stop=True)
            nc.vector.tensor_copy(srcB, pb)

            nc.sync.dma_start(who_d[0:64, :], whTh)
            nc.sync.dma_start(who_d[64:65, :], onesRh)
            nc.sync.dma_start(who_d[65:66, :], dstR)

        # ------------- branch post: whoT tiles (bf16 lhsT + fp32 dst) --
        def branch_post(whoF_d):
            whoTb = whoTp.tile([128, JT, 65], BF, tag="whoTb")
            dstc = whoTp.tile([128, JT, 1], FP, tag="dstc")
            for c8 in range(NCORES):
                wf = gchunk.tile([66, NB], BF, tag="gch")
                nc.sync.dma_start(wf, whoF_d[c8])
                ptt = psT.tile([128, 1024], BF, tag="pt")
                pth = ptt.rearrange("p (k n) -> p k n", k=8)
                for nt8 in range(NT):
                    nc.tensor.transpose(
                        pth[:, nt8, 0:66],
                        wf[:, nt8 * 128 : (nt8 + 1) * 128],
                        identb[:66, :66],
                    )
                jt0 = c8 * NT
                psum_copy(whoTb[:, jt0 : jt0 + 4, :], pth[:, 0:NT, 0:65])
                psum_copy(dstc[:, jt0 : jt0 + 4, :], pth[:, 0:NT, 65:66])
            return whoTb, dstc

        # ---------------- attention ----------------
        def attention(srcB, whoTb, dstc, adjT, fillers=(), start_jt=0):
            fillers = list(fillers)
            pa = psA.tile([65, NB], FP, tag="acc")
            for jt in range(JT):
                et = ebuf.tile([128, NB], FP, tag="et")
                nc.scalar.activation(
                    et, srcB, AF.Prelu,
                    bias=dstc[:, jt, :], scale=1.0, alpha=0.2,
                )
                at = ebuf.tile([128, NB], BF, tag="at")
                nc.scalar.activation(at, et, AF.Exp, bias=negC[:, 0:1])
                nc.vector.tensor_mul(at, at, adjT[:, jt, :])
                nc.tensor.matmul(
                    pa, whoTb[:, jt, :], at,
                    start=(jt == 0), stop=(jt == JT - 1),
                )
                if jt >= start_jt and fillers:
                    fillers.pop(0)()
            while fillers:
                fillers.pop(0)()
            ao = scr.tile([65, NB], FP, tag="sc")
            nc.vector.tensor_copy(ao, pa)
            rs = rows.tile([1, NB], FP, tag="rows")
            act_recip(rs, ao[64:65, :])
            pb = psMM.tile([64, NB], FP, tag="mm")
            nc.tensor.matmul(pb, ones1[:, :64], rs, start=True, stop=True)
            hp = scr.tile([64, NB], FP, tag="sc")
            nc.vector.tensor_mul(hp, ao[0:64, :], pb)
            return hp

        # ---------------- mlp ----------------
        def mlp(hp, ztag):
            x1 = x1p.tile([128, 2, NB], FP, tag="x1")
            for chk in range(2):
                pm = psMM.tile([128, NB], FP, tag="mm")
                nc.tensor.matmul(
                    pm, mw1_sb[:, chk * 128 : (chk + 1) * 128], hp,
                    start=True, stop=True,
                )
                nc.scalar.activation(
                    x1[:, chk, :], pm, AF.Relu, bias=b1_sb[:, chk : chk + 1]
                )
            pm2 = psMM.tile([128, NB], FP, tag="mm")
            for kt in range(2):
                nc.tensor.matmul(
                    pm2, mw2_sb[:, kt, :], x1[:, kt, :],
                    start=(kt == 0), stop=(kt == 1),
                )
            x2 = scr.tile([128, NB], FP, tag="sc")
            nc.scalar.activation(x2, pm2, AF.Relu, bias=b2_sb[:, 0:1])
            pm3 = psMM.tile([64, NB], FP, tag="mm")
            nc.tensor.matmul(pm3, mw3_sb, x2, start=True, stop=True)
            zT = longp.tile([64, NB], FP, tag=ztag)
            nc.scalar.activation(zT, pm3, AF.Identity, bias=b3_sb[:, 0:1])
            return zT

        # big transposed 0/1 matrices (fp16, exact)
        def bigT_group(dst, src_name, it, q):
            nat = natp.tile([128, 1024], BF, tag="nat", name="nat")
            nc.gpsimd.dma_start(
                nat,
                io[src_name][it * 128 : (it + 1) * 128,
                             q * 1024 : (q + 1) * 1024],
            )
            ptt = psT.tile([128, 1024], BF, tag="pt", name="ptt")
            for jl in range(8):
                nc.tensor.transpose(
                    ptt[:, jl * 128 : (jl + 1) * 128],
                    nat[:, jl * 128 : (jl + 1) * 128],
                    identb,
                )
            psum_copy(
                dst[:, q * 8 : (q + 1) * 8, it * 128 : (it + 1) * 128],
                ptt.rearrange("p (k n) -> p k n", k=8),
            )

        def build_bigT(dst, src_name, groups=None):
            for (it, q) in (groups or [(i, q) for i in range(NT) for q in range(4)]):
                bigT_group(dst, src_name, it, q)

        def dmlp(x, tagp):
            pm = psMM.tile([128, NB], FP, tag="mm", name="dpm")
            nc.tensor.matmul(pm, dw1_sb, x, start=True, stop=True)
            t = scr.tile([128, NB], FP, tag="sc", name="dt")
            nc.scalar.activation(t, pm, AF.Relu, bias=db1_sb[:, 0:1])
            pm2 = psMM.tile([64, NB], FP, tag="mm", name="dpm2")
            nc.tensor.matmul(pm2, dw2_sb, t, start=True, stop=True)
            c = longp.tile([64, NB], FP, tag=tagp, name="dmc")
            nc.scalar.activation(c, pm2, AF.Identity, bias=db2_sb[:, 0:1])
            return c

        # ================= schedule =================
        # tiny dependency-free collective up front absorbs comm setup cost
        warm_d = dram.tile([1, 64], FP, tag="warm_d")
        warmrow = const.tile([1, 64], FP, tag="warmrow", name="warmrow")
        nc.vector.memset(warmrow, 0.0)
        nc.sync.dma_start(warm_d[:], warmrow)
        warmF_d = dram.tile([NCORES, 1, 64], FP, tag="warmF_d")
        nc.gpsimd.collective_compute(
            "AllGather", ALU.bypass, replica_groups=RG,
            ins=[warm_d[:].opt()], outs=[warmF_d[:].opt()],
        )

        who1_d = dram.tile([66, NB], BF, tag="who1_d")
        who2_d = dram.tile([66, NB], BF, tag="who2_d")
        whoF1_d = dram.tile([NCORES, 66, NB], BF, tag="whoF1_d")
        whoF2_d = dram.tile([NCORES, 66, NB], BF, tag="whoF2_d")

        fT1 = fTp.tile([128, KT, NB], BF, tag="fT", name="fT1")
        srcB1 = srcp.tile([128, NB], FP, tag="srcB", name="srcB1")
        for it in range(NT):
            for q in range(3):
                feat_group(fT1, "feat", it, q)
        branch_tail(fT1, who1_d, srcB1)
        nc.gpsimd.collective_compute(
            "AllGather", ALU.bypass, replica_groups=RG,
            ins=[who1_d[:].opt()], outs=[whoF1_d[:].opt()],
        )

        fT2 = fTp.tile([128, KT, NB], BF, tag="fT", name="fT2")
        srcB2 = srcp.tile([128, NB], FP, tag="srcB", name="srcB2")
        for it in range(NT):
            for q in range(3):
                feat_group(fT2, "feat_a", it, q)
        branch_tail(fT2, who2_d, srcB2)
        nc.gpsimd.collective_compute(
            "AllGather", ALU.bypass, replica_groups=RG,
            ins=[who2_d[:].opt()], outs=[whoF2_d[:].opt()],
        )

        adjT = bigA.tile([128, JT, 512], BF, tag="adjT")
        gnT = bigG.tile([128, JT, 512], BF, tag="gnT")
        # q=0 of adjT up front (covers jt 0..7); rest threads through attn1
        build_bigT(adjT, "adj", [(i, 0) for i in range(NT)])

        whoTb1, dstc1 = branch_post(whoF1_d)
        adj_rest = [(i, q) for q in (1, 2, 3) for i in range(NT)]
        gn_groups = [(i, q) for i in range(NT) for q in range(4)]

        def mk_bigT(dst, name, it, q):
            return lambda: bigT_group(dst, name, it, q)

        hp1 = attention(
            srcB1, whoTb1, dstc1, adjT,
            fillers=[mk_bigT(adjT, "adj", it, q) for (it, q) in adj_rest]
            + [mk_bigT(gnT, "graph_neigh", it, q) for (it, q) in gn_groups],
        )
        zT = mlp(hp1, "zT")
        nc.sync.dma_start(io["hiden_emb_T"][:], zT)
        embT = longp.tile([64, NB], FP, tag="embT")
        nc.scalar.activation(embT, zT, AF.Relu)
        dm_e = dmlp(embT, "dme")

        # z AllGather (launch early; overlaps attention2)
        z_d = dram.tile([64, NB], FP, tag="z_d")
        nc.sync.dma_start(z_d[:], zT)
        zF_d = dram.tile([NCORES, 64, NB], FP, tag="zF_d")
        nc.gpsimd.collective_compute(
            "AllGather", ALU.bypass, replica_groups=RG,
            ins=[z_d[:].opt()], outs=[zF_d[:].opt()],
        )

        zfull = zfp.tile([128, JT, 64], BF, tag="zfull")
        embO = embp.tile([128, JT, 65], BF, tag="embO")
        nc.vector.memset(embO[:, :, 64:65], 1.0)

        def z_group(c8):
            zc = gchunk.tile([66, NB], FP, tag="gch", name="zc")
            nc.sync.dma_start(zc[:64, :], zF_d[c8])
            ptt = psT.tile([128, 1024], BF, tag="pt", name="zpt")
            ptf = ptt.bitcast(FP).rearrange("p (k n) -> p k n", k=4)
            for nt8 in range(NT):
                nc.tensor.transpose(
                    ptf[:, nt8, 0:64],
                    zc[:64, nt8 * 128 : (nt8 + 1) * 128],
                    identf[:64, :64],
                )
            jt0 = c8 * NT
            psum_copy(zfull[:, jt0 : jt0 + 4, :], ptf[:, :, 0:64])
            nc.vector.tensor_relu(embO[:, jt0 : jt0 + 4, 0:64], ptf[:, :, 0:64])

        whoTb2, dstc2 = branch_post(whoF2_d)
        hp2 = attention(
            srcB2, whoTb2, dstc2, adjT,
            fillers=[(lambda c8_=c8: z_group(c8_)) for c8 in range(NCORES)],
            start_jt=20,
        )
        zaT = mlp(hp2, "zaT")
        embaT = longp.tile([64, NB], FP, tag="embaT")
        nc.scalar.activation(embaT, zaT, AF.Relu)
        dm_ea = dmlp(embaT, "dmea")

        za_d = dram.tile([64, NB], FP, tag="za_d")
        nc.sync.dma_start(za_d[:], zaT)
        zaF_d = dram.tile([NCORES, 64, NB], FP, tag="zaF_d")
        nc.gpsimd.collective_compute(
            "AllGather", ALU.bypass, replica_groups=RG,
            ins=[za_d[:].opt()], outs=[zaF_d[:].opt()],
        )

        # az = adj @ z (transposed), then h = az @ W2 (natural rows)
        paz = psA.tile([64, NB], FP, tag="acc")
        for jt in range(JT):
            nc.tensor.matmul(
                paz, zfull[:, jt, :], adjT[:, jt, :],
                start=(jt == 0), stop=(jt == JT - 1),
            )
        azT = longp.tile([64, NB], BF, tag="azT")
        nc.vector.tensor_copy(azT, paz)

        MCW = 500
        for it in range(NT):
            for mh in range(3):
                hs = hstp.tile([128, 2 * MCW], FP, tag="hst")
                for ml in range(2):
                    mc = mh * 2 + ml
                    pm = psMM.tile([128, MCW], FP, tag="mm")
                    nc.tensor.matmul(
                        pm,
                        azT[:, it * 128 : (it + 1) * 128],
                        w2_sb[:, mc * MCW : (mc + 1) * MCW],
                        start=True, stop=True,
                    )
                    psum_copy(hs[:, ml * MCW : (ml + 1) * MCW], pm)
                nc.sync.dma_start(
                    io["h"][it * 128 : (it + 1) * 128,
                            mh * 2 * MCW : (mh + 1) * 2 * MCW],
                    hs,
                )

        # readout pass 1 over resident gn^T tiles (N=512 matmuls)
        pro1 = psA.tile([65, NB], FP, tag="acc")
        for jt in range(JT):
            nc.tensor.matmul(
                pro1, embO[:, jt, :], gnT[:, jt, :],
                start=(jt == 0), stop=(jt == JT - 1),
            )

        # shared: 1/rowsum(graph_neigh) (row 64 of either pro)
        rsg = rows.tile([1, NB], FP, tag="rows")
        rog = scr.tile([65, NB], FP, tag="sc", name="rog")
        nc.vector.tensor_copy(rog, pro1)
        act_recip(rsg, rog[64:65, :])
        pbg = psMM.tile([64, NB], FP, tag="mm")
        nc.tensor.matmul(pbg, ones1[:, :64], rsg, start=True, stop=True)
        rsgb = longp.tile([64, NB], FP, tag="rsgb")
        nc.vector.tensor_copy(rsgb, pbg)

        def readout_finish(ro64, gtag):
            gpre = scr.tile([64, NB], FP, tag="sc", name="gpre")
            nc.vector.tensor_mul(gpre, ro64, rsgb)
            sq = scr.tile([64, NB], FP, tag="sc", name="sq")
            nc.scalar.activation(sq, gpre, AF.Square)
            pn = psR.tile([1, NB], FP, tag="row", name="pn")
            nc.tensor.matmul(pn, ones64, sq, start=True, stop=True)
            rn = rows.tile([1, NB], FP, tag="rows", name="rn")
            act_recip(rn, pn, scale=-0.5, clamp=1e-24)
            pb2 = psMM.tile([64, NB], FP, tag="mm", name="pb2")
            nc.tensor.matmul(pb2, ones1[:, :64], rn, start=True, stop=True)
            gg = scr.tile([64, NB], FP, tag="sc", name="gg")
            nc.vector.tensor_mul(gg, gpre, pb2)
            g = longp.tile([64, NB], FP, tag=gtag, name="g")
            nc.scalar.activation(g, gg, AF.Sigmoid)
            return g

        def bilinear(x, y, out_ap):
            pu = psMM.tile([64, NB], FP, tag="mm", name="pu")
            nc.tensor.matmul(pu, bilwT_sb, y, start=True, stop=True)
            p = scr.tile([64, NB], FP, tag="sc", name="bp")
            nc.vector.tensor_mul(p, x, pu)
            pr = psR.tile([1, NB], FP, tag="row", name="pr")
            nc.tensor.matmul(pr, ones64, p, start=True, stop=True)
            nc.scalar.activation(out_ap, pr, AF.Identity, bias=bilb_sb[:, 0:1])

        # g1 branch finishes while the g2 branch still waits on za-AG
        g1 = readout_finish(rog[0:64, :], "g1")
        dm_g = dmlp(g1, "dmg")
        r00 = longp.tile([1, NB], FP, tag="r00")
        r01 = longp.tile([1, NB], FP, tag="r01")
        bilinear(dm_e, dm_g, r00)
        bilinear(dm_ea, dm_g, r01)
        nc.sync.dma_start(io["ret_T"][0:1, :], r00)
        nc.sync.dma_start(io["ret_T"][1:2, :], r01)

        # embaO tiles, then second readout pass
        embaO = embp.tile([128, JT, 65], BF, tag="embO")
        nc.vector.memset(embaO[:, :, 64:65], 1.0)
        for c8 in range(NCORES):
            zc = gchunk.tile([66, NB], FP, tag="gch")
            nc.sync.dma_start(zc[:64, :], zaF_d[c8])
            ptt = psT.tile([128, 1024], BF, tag="pt")
            ptf = ptt.bitcast(FP).rearrange("p (k n) -> p k n", k=4)
            for nt8 in range(NT):
                nc.tensor.transpose(
                    ptf[:, nt8, 0:64],
                    zc[:64, nt8 * 128 : (nt8 + 1) * 128],
                    identf[:64, :64],
                )
            jt0 = c8 * NT
            nc.vector.tensor_relu(embaO[:, jt0 : jt0 + 4, 0:64], ptf[:, :, 0:64])

        pro2 = psA.tile([65, NB], FP, tag="acc")
        for jt in range(JT):
            nc.tensor.matmul(
                pro2, embaO[:, jt, :], gnT[:, jt, :],
                start=(jt == 0), stop=(jt == JT - 1),
            )

        ro2s = scr.tile([65, NB], FP, tag="sc", name="ro2s")
        nc.vector.tensor_copy(ro2s, pro2)
        g2 = readout_finish(ro2s[0:64, :], "g2")
        dm_ga = dmlp(g2, "dmga")
        r10 = longp.tile([1, NB], FP, tag="r10")
        r11 = longp.tile([1, NB], FP, tag="r11")
        bilinear(dm_ea, dm_ga, r10)
        bilinear(dm_e, dm_ga, r11)
        nc.sync.dma_start(io["ret_a_T"][0:1, :], r10)
        nc.sync.dma_start(io["ret_a_T"][1:2, :], r11)


_CACHED = {}


def _get_program():
    if "nc" not in _CACHED:
        nc = bass.Bass(num_devices=NCORES)
        _build(nc)
        _split_waits(nc)
        _CACHED["nc"] = nc
    return _CACHED["nc"]


def run(inputs, **kwargs):
    nc = _get_program()
    w_names = [
        "weight1", "weight2", "att_W", "a_src", "a_dst",
        "mlp_W1", "mlp_b1", "mlp_W2", "mlp_b2", "mlp_W3", "mlp_b3",
        "dmlp_W1", "dmlp_b1", "dmlp_W2", "dmlp_b2", "bil_W", "bil_b",
    ]
    ws = {k: np.ascontiguousarray(np.asarray(inputs[k], dtype=np.float32))
          for k in w_names}
    in_maps = []
    for c in range(NCORES):
        sl = slice(c * NB, (c + 1) * NB)
        m = dict(ws)
        for k in ("feat", "feat_a", "adj", "graph_neigh"):
            m[k] = np.ascontiguousarray(
                np.asarray(inputs[k], dtype=np.float32)[sl])
        in_maps.append(m)

    res = run_bass_kernel_spmd(nc, in_maps, core_ids=list(range(NCORES)), **kwargs)
    outs = res.results
    hiden_emb = np.concatenate([o["hiden_emb_T"].T for o in outs], axis=0)
    h = np.concatenate([o["h"] for o in outs], axis=0)
    ret = np.concatenate([o["ret_T"].T for o in outs], axis=0)
    ret_a = np.concatenate([o["ret_a_T"].T for o in outs], axis=0)
    return (hiden_emb, h, ret, ret_a), res


def kernel(**inputs):
    out, _ = run(inputs)
    return out


# revision 29
# speedup vs baseline: 1.1096x; 1.0605x over previous
"""Trainium2 Bass kernel for nn_EncoderNoResidual (GNN message passing).

Row-parallel across 8 NeuronCores: core c owns nodes [c*512, (c+1)*512).
Activations kept in transposed [feature, node] layout on-chip so the PE
contracts over the partition dim.  adj / graph_neigh row-blocks are
cast to bf16 in-flight (SWDGE DMA cast; 0/1 values are exact in bf16),
transposed on-chip once (batched PE transposes) and kept resident in
SBUF.  h = adj @ (z @ W2) is computed as (adj @ z) @ W2 (associativity:
3.7 GFLOP instead of 100).  Softmax skips max-subtraction (logits are
O(10)); non-edges are masked by multiplying exp(e) with the 0/1
adjacency, matching the reference's -9e15 mask.  PE-heavy matmuls run
in bf16 (fp32 matmul is 2-pass on TRN2); accumulation stays fp32.

Cross-core traffic: one merged AllGather of [2, 66, 512] (Wh|1|dst for
both attention branches) + two [64, 512] AllGathers (z, z_a).
Outputs are per-core row blocks assembled on host.
"""

import sys

sys.path.insert(0, "/opt/trn_rl_repo")

import numpy as np

import concourse.bass as bass
import concourse.tile as tile
from concourse import mybir
from concourse.bass_utils import run_bass_kernel_spmd
from concourse.masks import make_identity
import bass_rust

FP = mybir.dt.float32
BF = mybir.dt.float16  # 16-bit compute dtype (fp16: 11-bit mantissa)
AF = mybir.ActivationFunctionType
ALU = mybir.AluOpType

N = 4096
F_IN = 3000
F_OUT = 64
NCORES = 8
NB = N // NCORES          # 512 nodes per core
NT = NB // 128            # 4 node-tiles per core
JT = N // 128             # 32 j-tiles globally
KT = (F_IN + 127) // 128  # 24 k-tiles over F_IN (last = 56)
LAST_K = F_IN - 128 * (KT - 1)


def _split_waits(nc, max_waits=1):
    """This container's walrus accepts only ONE sync wait per instruction.
    Move excess waits onto NoOps inserted just before the offending
    instruction on the same engine (streams are in-order, so this is
    semantically identical)."""
    n = 0
    for fn in nc.m.functions:
        for bb in fn.blocks:
            out = []
            changed = False
            for ins in list(bb.instructions):
                si = ins.sync_info
                waits = list(si.on_wait) if si is not None else []
                if len(waits) > max_waits:
                    changed = True
                    n += 1
                    head, rest = waits[:-max_waits], waits[-max_waits:]
                    for i in range(0, len(head), max_waits):
                        out.append(
                            mybir.InstNoOp(
                                name=f"{ins.name}_ws{i}",
                                engine=ins.engine,
                                ins=[],
                                outs=[],
                                sync_info=bass_rust.SyncInfo(
                                    on_wait=head[i : i + max_waits], on_update=[]
                                ),
                            )
                        )
                    ins.sync_info = bass_rust.SyncInfo(
                        on_wait=rest, on_update=list(si.on_update)
                    )
                out.append(ins)
            if changed:
                bb.instructions = out
    return n


def _build(nc):
    io = {}
    def din(name, shape):
        io[name] = nc.dram_tensor(name, shape, FP, kind="ExternalInput")
    def dout(name, shape):
        io[name] = nc.dram_tensor(name, shape, FP, kind="ExternalOutput")

    din("feat", [NB, F_IN]); din("feat_a", [NB, F_IN])
    din("adj", [NB, N]); din("graph_neigh", [NB, N])
    din("weight1", [F_IN, F_OUT]); din("weight2", [F_OUT, F_IN])
    din("att_W", [F_OUT, F_OUT]); din("a_src", [F_OUT, 1]); din("a_dst", [F_OUT, 1])
    din("mlp_W1", [64, 256]); din("mlp_b1", [256])
    din("mlp_W2", [256, 128]); din("mlp_b2", [128])
    din("mlp_W3", [128, 64]); din("mlp_b3", [64])
    din("dmlp_W1", [64, 128]); din("dmlp_b1", [128])
    din("dmlp_W2", [128, 64]); din("dmlp_b2", [64])
    din("bil_W", [64, 64]); din("bil_b", [1])
    dout("hiden_emb_T", [64, NB]); dout("h", [NB, F_IN])
    dout("ret_T", [2, NB]); dout("ret_a_T", [2, NB])

    with tile.TileContext(nc) as tc:
        _graph(nc, tc, io)
    return nc


def _graph(nc, tc, io):
    from contextlib import ExitStack

    RG = [list(range(NCORES))]

    ctx = ExitStack()
    with ctx:
        const = ctx.enter_context(tc.tile_pool(name="const", bufs=1))
        bigA = ctx.enter_context(tc.tile_pool(name="bigA", bufs=1))
        bigG = ctx.enter_context(tc.tile_pool(name="bigG", bufs=1))
        natp = ctx.enter_context(tc.tile_pool(name="natp", bufs=3))
        fTp = ctx.enter_context(tc.tile_pool(name="fTp", bufs=1))
        whoTp = ctx.enter_context(tc.tile_pool(name="whoTp", bufs=1))
        embp = ctx.enter_context(tc.tile_pool(name="embp", bufs=2))
        zfp = ctx.enter_context(tc.tile_pool(name="zfp", bufs=1))
        gchunk = ctx.enter_context(tc.tile_pool(name="gchunk", bufs=2))
        ebuf = ctx.enter_context(tc.tile_pool(name="ebuf", bufs=3))
        srcp = ctx.enter_context(tc.tile_pool(name="srcp", bufs=2))
        hstp = ctx.enter_context(tc.tile_pool(name="hstp", bufs=2))
        longp = ctx.enter_context(tc.tile_pool(name="longp", bufs=1))
        scr = ctx.enter_context(tc.tile_pool(name="scr", bufs=4))
        x1p = ctx.enter_context(tc.tile_pool(name="x1p", bufs=1))
        rows = ctx.enter_context(tc.tile_pool(name="rows", bufs=3))
        dram = ctx.enter_context(tc.tile_pool(name="dram", bufs=1, space="DRAM"))
        psT = ctx.enter_context(tc.tile_pool(name="psT", bufs=2, space="PSUM"))
        psMM = ctx.enter_context(tc.tile_pool(name="psMM", bufs=3, space="PSUM"))
        psA = ctx.enter_context(tc.tile_pool(name="psA", bufs=2, space="PSUM"))
        psR = ctx.enter_context(tc.tile_pool(name="psR", bufs=1, space="PSUM"))

        # ---------------- constants / weights ----------------
        identb = const.tile([128, 128], BF, tag="identb")
        make_identity(nc, identb)
        identf = const.tile([128, 128], FP, tag="identf")
        make_identity(nc, identf)
        ones1 = const.tile([1, 128], FP, tag="ones1")
        nc.vector.memset(ones1, 1.0)
        ones64 = const.tile([64, 1], FP, tag="ones64")
        nc.vector.memset(ones64, 1.0)
        onesR = const.tile([1, NB], FP, tag="onesR")
        nc.vector.memset(onesR, 1.0)
        onesRh = const.tile([1, NB], BF, tag="onesRh")
        nc.vector.memset(onesRh, 1.0)
        negC = const.tile([128, 1], FP, tag="negC")
        nc.vector.memset(negC, -8.0)  # exp(e-8): keeps A in fp16 range; cancels in softmax norm

        # bf16 weights via SWDGE cast DMA
        w1_sb = const.tile([128, KT, F_OUT], BF, tag="w1")
        nc.gpsimd.dma_start(
            w1_sb[:, : KT - 1, :],
            io["weight1"][: 128 * (KT - 1)].rearrange("(t p) f -> p t f", p=128),
        )
        nc.gpsimd.dma_start(w1_sb[:LAST_K, KT - 1, :], io["weight1"][128 * (KT - 1):])
        w2_sb = const.tile([64, F_IN], BF, tag="w2")
        nc.gpsimd.dma_start(w2_sb, io["weight2"][:])

        attw_sb = const.tile([64, 64], FP, tag="attw")
        nc.sync.dma_start(attw_sb, io["att_W"][:])
        asrc_sb = const.tile([64, 1], FP, tag="asrc")
        nc.sync.dma_start(asrc_sb, io["a_src"][:])
        adst_sb = const.tile([64, 1], FP, tag="adst")
        nc.sync.dma_start(adst_sb, io["a_dst"][:])
        mw1_sb = const.tile([64, 256], FP, tag="mw1")
        nc.sync.dma_start(mw1_sb, io["mlp_W1"][:])
        mw2_sb = const.tile([128, 2, 128], FP, tag="mw2")
        nc.sync.dma_start(mw2_sb, io["mlp_W2"][:].rearrange("(t p) f -> p t f", p=128))
        mw3_sb = const.tile([128, 64], FP, tag="mw3")
        nc.sync.dma_start(mw3_sb, io["mlp_W3"][:])
        dw1_sb = const.tile([64, 128], FP, tag="dw1")
        nc.sync.dma_start(dw1_sb, io["dmlp_W1"][:])
        dw2_sb = const.tile([128, 64], FP, tag="dw2")
        nc.sync.dma_start(dw2_sb, io["dmlp_W2"][:])
        bilw_sb = const.tile([64, 64], FP, tag="bilw")
        nc.sync.dma_start(bilw_sb, io["bil_W"][:])

        b1_sb = const.tile([128, 2], FP, tag="b1")
        nc.sync.dma_start(b1_sb, io["mlp_b1"][:].rearrange("(t p) -> p t", p=128))
        b2_sb = const.tile([128, 1], FP, tag="b2")
        nc.sync.dma_start(b2_sb, io["mlp_b2"][:].rearrange("(p x) -> p x", x=1))
        b3_sb = const.tile([64, 1], FP, tag="b3")
        nc.sync.dma_start(b3_sb, io["mlp_b3"][:].rearrange("(p x) -> p x", x=1))
        db1_sb = const.tile([128, 1], FP, tag="db1")
        nc.sync.dma_start(db1_sb, io["dmlp_b1"][:].rearrange("(p x) -> p x", x=1))
        db2_sb = const.tile([64, 1], FP, tag="db2")
        nc.sync.dma_start(db2_sb, io["dmlp_b2"][:].rearrange("(p x) -> p x", x=1))
        bilb_sb = const.tile([1, 1], FP, tag="bilb")
        nc.sync.dma_start(bilb_sb, io["bil_b"][:].rearrange("(p x) -> p x", x=1))

        pt0 = psT.tile([128, 1024], BF, tag="pt")
        ptf0 = pt0.bitcast(FP)  # reuse the bank for one fp32 transpose
        nc.tensor.transpose(ptf0[:64, :64], bilw_sb, identf[:64, :64])
        bilwT_sb = const.tile([64, 64], FP, tag="bilwT")
        nc.vector.tensor_copy(bilwT_sb, ptf0[:64, :64])

        def psum_copy(dst, src):
            nc.vector.tensor_copy(dst, src)

        def act_recip(out, in_, scale=-1.0, clamp=None):
            # 1/x (scale=-1) or 1/sqrt(x) (scale=-0.5) via exp(scale*ln(x));
            # Ln+Exp live in the same ACT table set as Prelu/Relu.
            t = rows.tile([1, NB], FP, tag="rows", name="lnf")
            if clamp is not None:
                nc.vector.tensor_scalar_max(t, in_, clamp)
                nc.scalar.activation(t, t, AF.Ln)
            else:
                nc.scalar.activation(t, in_, AF.Ln)
            nc.scalar.activation(out, t, AF.Exp, scale=scale)

        # ------------- branch pre: feat -> h1T, who rows in DRAM -------
        def feat_group(fT, fname, it, q):
            c0 = q * 1024
            cw = min(1024, F_IN - c0)
            nk = (cw + 127) // 128
            nat = natp.tile([128, 1024], BF, tag="nat", name="fnat")
            nc.gpsimd.dma_start(
                nat[:, :cw],
                io[fname][it * 128 : (it + 1) * 128, c0 : c0 + cw],
            )
            ptt = psT.tile([128, 1024], BF, tag="pt", name="fptt")
            for kl in range(nk):
                kt = q * 8 + kl
                kc = LAST_K if kt == KT - 1 else 128
                nc.tensor.transpose(
                    ptt[:kc, kl * 128 : (kl + 1) * 128],
                    nat[:, kl * 128 : kl * 128 + kc],
                    identb,
                )
            psum_copy(
                fT[:, q * 8 : q * 8 + nk, it * 128 : (it + 1) * 128],
                ptt.rearrange("p (k n) -> p k n", k=8)[:, :nk, :],
            )

        def branch_tail(fT, who_d, srcB):
            h1T = scr.tile([64, NB], FP, tag="sc")
            pm = psMM.tile([64, NB], FP, tag="mm")
            for kt in range(KT):
                kc = LAST_K if kt == KT - 1 else 128
                nc.tensor.matmul(
                    pm, w1_sb[:kc, kt, :], fT[:kc, kt, :],
                    start=(kt == 0), stop=(kt == KT - 1),
                )
            nc.vector.tensor_copy(h1T, pm)

            # who rows: 0:64 Wh^T = att_W.T @ h1^T ; 64 ones ; 65 dst^T
            whT = scr.tile([64, NB], FP, tag="sc")
            whTh = scr.tile([64, NB], BF, tag="sch", bufs=2)
            pw = psMM.tile([64, NB], FP, tag="mm")
            nc.tensor.matmul(pw, attw_sb, h1T, start=True, stop=True)
            nc.vector.tensor_copy(whT, pw)
            nc.scalar.copy(whTh, pw)
            pd = psR.tile([1, NB], FP, tag="row")
            nc.tensor.matmul(pd, adst_sb, whT, start=True, stop=True)
            dstR = rows.tile([1, NB], BF, tag="rows")
            nc.vector.tensor_copy(dstR, pd)

            ps = psR.tile([1, NB], FP, tag="row")
            nc.tensor.matmul(ps, asrc_sb, whT, start=True, stop=True)
            src_sb = rows.tile([1, NB], FP, tag="rows")
            nc.vector.tensor_copy(src_sb, ps)
            pb = psMM.tile([128, NB], FP, tag="mm")
            nc.tensor.matmul(pb, ones1, src_sb, start=True, stop=True)
            nc.vector.tensor_copy(srcB, pb)

            nc.sync.dma_start(who_d[0:64, :], whTh)
            nc.sync.dma_start(who_d[64:65, :], onesRh)
            nc.sync.dma_start(who_d[65:66, :], dstR)

        # ------------- branch post: whoT tiles (bf16 lhsT + fp32 dst) --
        def branch_post(whoF_d):
            whoTb = whoTp.tile([128, JT, 65], BF, tag="whoTb")
            dstc = whoTp.tile([128, JT, 1], FP, tag="dstc")
            for c8 in range(NCORES):
                wf = gchunk.tile([66, NB], BF, tag="gch")
                nc.sync.dma_start(wf, whoF_d[c8])
                ptt = psT.tile([128, 1024], BF, tag="pt")
                pth = ptt.rearrange("p (k n) -> p k n", k=8)
                for nt8 in range(NT):
                    nc.tensor.transpose(
                        pth[:, nt8, 0:66],
                        wf[:, nt8 * 128 : (nt8 + 1) * 128],
                        identb[:66, :66],
                    )
                jt0 = c8 * NT
                psum_copy(whoTb[:, jt0 : jt0 + 4, :], pth[:, 0:NT, 0:65])
                psum_copy(dstc[:, jt0 : jt0 + 4, :], pth[:, 0:NT, 65:66])
            return whoTb, dstc

        # ---------------- attention ----------------
        def attention(srcB, whoTb, dstc, adjT, fillers=(), start_jt=0,
                      dve_lrelu_mod=0):
            fillers = list(fillers)
            pa = psA.tile([65, NB], FP, tag="acc")
            for jt in range(JT):
                et = ebuf.tile([128, NB], FP, tag="et")
                if dve_lrelu_mod and jt % dve_lrelu_mod == 0:
                    t2 = ebuf.tile([128, NB], FP, tag="et2", bufs=2)
                    nc.vector.tensor_scalar(
                        t2, srcB, dstc[:, jt, :], 0.2, ALU.add, ALU.mult)
                    nc.vector.tensor_scalar_add(et, srcB, dstc[:, jt, :])
                    nc.vector.tensor_max(et, et, t2)
                else:
                    nc.scalar.activation(
                        et, srcB, AF.Prelu,
                        bias=dstc[:, jt, :], scale=1.0, alpha=0.2,
                    )
                at = ebuf.tile([128, NB], BF, tag="at")
                nc.scalar.activation(at, et, AF.Exp, bias=negC[:, 0:1])
                nc.vector.tensor_mul(at, at, adjT[:, jt, :])
                nc.tensor.matmul(
                    pa, whoTb[:, jt, :], at,
                    start=(jt == 0), stop=(jt == JT - 1),
                )
                if jt >= start_jt and fillers:
                    fillers.pop(0)()
            while fillers:
                fillers.pop(0)()
            ao = scr.tile([65, NB], FP, tag="sc")
            nc.vector.tensor_copy(ao, pa)
            rs = rows.tile([1, NB], FP, tag="rows")
            act_recip(rs, ao[64:65, :])
            pb = psMM.tile([64, NB], FP, tag="mm")
            nc.tensor.matmul(pb, ones1[:, :64], rs, start=True, stop=True)
            hp = scr.tile([64, NB], FP, tag="sc")
            nc.vector.tensor_mul(hp, ao[0:64, :], pb)
            return hp

        # ---------------- mlp ----------------
        def mlp(hp, ztag):
            x1 = x1p.tile([128, 2, NB], FP, tag="x1")
            for chk in range(2):
                pm = psMM.tile([128, NB], FP, tag="mm")
                nc.tensor.matmul(
                    pm, mw1_sb[:, chk * 128 : (chk + 1) * 128], hp,
                    start=True, stop=True,
                )
                nc.scalar.activation(
                    x1[:, chk, :], pm, AF.Relu, bias=b1_sb[:, chk : chk + 1]
                )
            pm2 = psMM.tile([128, NB], FP, tag="mm")
            for kt in range(2):
                nc.tensor.matmul(
                    pm2, mw2_sb[:, kt, :], x1[:, kt, :],
                    start=(kt == 0), stop=(kt == 1),
                )
            x2 = scr.tile([128, NB], FP, tag="sc")
            nc.scalar.activation(x2, pm2, AF.Relu, bias=b2_sb[:, 0:1])
            pm3 = psMM.tile([64, NB], FP, tag="mm")
            nc.tensor.matmul(pm3, mw3_sb, x2, start=True, stop=True)
            zT = longp.tile([64, NB], FP, tag=ztag)
            nc.scalar.activation(zT, pm3, AF.Identity, bias=b3_sb[:, 0:1])
            return zT

        # big transposed 0/1 matrices (fp16, exact)
        def bigT_group(dst, src_name, it, q):
            nat = natp.tile([128, 1024], BF, tag="nat", name="nat")
            nc.gpsimd.dma_start(
                nat,
                io[src_name][it * 128 : (it + 1) * 128,
                             q * 1024 : (q + 1) * 1024],
            )
            ptt = psT.tile([128, 1024], BF, tag="pt", name="ptt")
            for jl in range(8):
                nc.tensor.transpose(
                    ptt[:, jl * 128 : (jl + 1) * 128],
                    nat[:, jl * 128 : (jl + 1) * 128],
                    identb,
                )
            psum_copy(
                dst[:, q * 8 : (q + 1) * 8, it * 128 : (it + 1) * 128],
                ptt.rearrange("p (k n) -> p k n", k=8),
            )

        def build_bigT(dst, src_name, groups=None):
            for (it, q) in (groups or [(i, q) for i in range(NT) for q in range(4)]):
                bigT_group(dst, src_name, it, q)

        def dmlp(x, tagp):
            pm = psMM.tile([128, NB], FP, tag="mm", name="dpm")
            nc.tensor.matmul(pm, dw1_sb, x, start=True, stop=True)
            t = scr.tile([128, NB], FP, tag="sc", name="dt")
            nc.scalar.activation(t, pm, AF.Relu, bias=db1_sb[:, 0:1])
            pm2 = psMM.tile([64, NB], FP, tag="mm", name="dpm2")
            nc.tensor.matmul(pm2, dw2_sb, t, start=True, stop=True)
            c = longp.tile([64, NB], FP, tag=tagp, name="dmc")
            nc.scalar.activation(c, pm2, AF.Identity, bias=db2_sb[:, 0:1])
            return c

        # ================= schedule =================
        # tiny dependency-free collective up front absorbs comm setup cost
        warm_d = dram.tile([1, 64], FP, tag="warm_d")
        warmrow = const.tile([1, 64], FP, tag="warmrow", name="warmrow")
        nc.vector.memset(warmrow, 0.0)
        nc.sync.dma_start(warm_d[:], warmrow)
        warmF_d = dram.tile([NCORES, 1, 64], FP, tag="warmF_d")
        nc.gpsimd.collective_compute(
            "AllGather", ALU.bypass, replica_groups=RG,
            ins=[warm_d[:].opt()], outs=[warmF_d[:].opt()],
        )

        who1_d = dram.tile([66, NB], BF, tag="who1_d")
        who2_d = dram.tile([66, NB], BF, tag="who2_d")
        whoF1_d = dram.tile([NCORES, 66, NB], BF, tag="whoF1_d")
        whoF2_d = dram.tile([NCORES, 66, NB], BF, tag="whoF2_d")

        fT1 = fTp.tile([128, KT, NB], BF, tag="fT", name="fT1")
        srcB1 = srcp.tile([128, NB], FP, tag="srcB", name="srcB1")
        for it in range(NT):
            for q in range(3):
                feat_group(fT1, "feat", it, q)
        branch_tail(fT1, who1_d, srcB1)
        nc.gpsimd.collective_compute(
            "AllGather", ALU.bypass, replica_groups=RG,
            ins=[who1_d[:].opt()], outs=[whoF1_d[:].opt()],
        )

        fT2 = fTp.tile([128, KT, NB], BF, tag="fT", name="fT2")
        srcB2 = srcp.tile([128, NB], FP, tag="srcB", name="srcB2")
        for it in range(NT):
            for q in range(3):
                feat_group(fT2, "feat_a", it, q)
        branch_tail(fT2, who2_d, srcB2)
        nc.gpsimd.collective_compute(
            "AllGather", ALU.bypass, replica_groups=RG,
            ins=[who2_d[:].opt()], outs=[whoF2_d[:].opt()],
        )

        adjT = bigA.tile([128, JT, 512], BF, tag="adjT")
        gnT = bigG.tile([128, JT, 512], BF, tag="gnT")
        # q=0 of adjT up front (covers jt 0..7); rest threads through attn1
        build_bigT(adjT, "adj", [(i, 0) for i in range(NT)])

        whoTb1, dstc1 = branch_post(whoF1_d)
        adj_rest = [(i, q) for q in (1, 2, 3) for i in range(NT)]
        gn_groups = [(i, q) for i in range(NT) for q in range(4)]

        def mk_bigT(dst, name, it, q):
            return lambda: bigT_group(dst, name, it, q)

        hp1 = attention(
            srcB1, whoTb1, dstc1, adjT,
            fillers=[mk_bigT(adjT, "adj", it, q) for (it, q) in adj_rest]
            + [mk_bigT(gnT, "graph_neigh", it, q) for (it, q) in gn_groups],
        )
        zT = mlp(hp1, "zT")
        nc.sync.dma_start(io["hiden_emb_T"][:], zT)
        embT = longp.tile([64, NB], FP, tag="embT")
        nc.scalar.activation(embT, zT, AF.Relu)
        dm_e = dmlp(embT, "dme")

        # z AllGather (launch early; overlaps attention2)
        z_d = dram.tile([64, NB], FP, tag="z_d")
        nc.sync.dma_start(z_d[:], zT)
        zF_d = dram.tile([NCORES, 64, NB], FP, tag="zF_d")
        nc.gpsimd.collective_compute(
            "AllGather", ALU.bypass, replica_groups=RG,
            ins=[z_d[:].opt()], outs=[zF_d[:].opt()],
        )

        zfull = zfp.tile([128, JT, 64], BF, tag="zfull")
        embO = embp.tile([128, JT, 65], BF, tag="embO")
        nc.vector.memset(embO[:, :, 64:65], 1.0)

        def z_group(c8):
            zc = gchunk.tile([66, NB], FP, tag="gch", name="zc")
            nc.sync.dma_start(zc[:64, :], zF_d[c8])
            ptt = psT.tile([128, 1024], BF, tag="pt", name="zpt")
            ptf = ptt.bitcast(FP).rearrange("p (k n) -> p k n", k=4)
            for nt8 in range(NT):
                nc.tensor.transpose(
                    ptf[:, nt8, 0:64],
                    zc[:64, nt8 * 128 : (nt8 + 1) * 128],
                    identf[:64, :64],
                )
            jt0 = c8 * NT
            psum_copy(zfull[:, jt0 : jt0 + 4, :], ptf[:, :, 0:64])
            nc.vector.tensor_relu(embO[:, jt0 : jt0 + 4, 0:64], ptf[:, :, 0:64])

        whoTb2, dstc2 = branch_post(whoF2_d)
        hp2 = attention(
            srcB2, whoTb2, dstc2, adjT,
            fillers=[(lambda c8_=c8: z_group(c8_)) for c8 in range(NCORES)],
            start_jt=20, dve_lrelu_mod=2,
        )
        zaT = mlp(hp2, "zaT")
        embaT = longp.tile([64, NB], FP, tag="embaT")
        nc.scalar.activation(embaT, zaT, AF.Relu)
        dm_ea = dmlp(embaT, "dmea")

        za_d = dram.tile([64, NB], FP, tag="za_d")
        nc.sync.dma_start(za_d[:], zaT)
        zaF_d = dram.tile([NCORES, 64, NB], FP, tag="zaF_d")
        nc.gpsimd.collective_compute(
            "AllGather", ALU.bypass, replica_groups=RG,
            ins=[za_d[:].opt()], outs=[zaF_d[:].opt()],
        )

        # az = adj @ z (transposed), then h = az @ W2 (natural rows)
        paz = psA.tile([64, NB], FP, tag="acc")
        for jt in range(JT):
            nc.tensor.matmul(
                paz, zfull[:, jt, :], adjT[:, jt, :],
                start=(jt == 0), stop=(jt == JT - 1),
            )
        azT = longp.tile([64, NB], BF, tag="azT")
        nc.vector.tensor_copy(azT, paz)

        MCW = 500
        for it in range(NT):
            for mh in range(3):
                hs = hstp.tile([128, 2 * MCW], FP, tag="hst")
                for ml in range(2):
                    mc = mh * 2 + ml
                    pm = psMM.tile([128, MCW], FP, tag="mm")
                    nc.tensor.matmul(
                        pm,
                        azT[:, it * 128 : (it + 1) * 128],
                        w2_sb[:, mc * MCW : (mc + 1) * MCW],
                        start=True, stop=True,
                    )
                    psum_copy(hs[:, ml * MCW : (ml + 1) * MCW], pm)
                nc.sync.dma_start(
                    io["h"][it * 128 : (it + 1) * 128,
                            mh * 2 * MCW : (mh + 1) * 2 * MCW],
                    hs,
                )

        # readout pass 1 over resident gn^T tiles (N=512 matmuls)
        pro1 = psA.tile([65, NB], FP, tag="acc")
        for jt in range(JT):
            nc.tensor.matmul(
                pro1, embO[:, jt, :], gnT[:, jt, :],
                start=(jt == 0), stop=(jt == JT - 1),
            )

        # shared: 1/rowsum(graph_neigh) (row 64 of either pro)
        rsg = rows.tile([1, NB], FP, tag="rows")
        rog = scr.tile([65, NB], FP, tag="sc", name="rog")
        nc.vector.tensor_copy(rog, pro1)
        act_recip(rsg, rog[64:65, :])
        pbg = psMM.tile([64, NB], FP, tag="mm")
        nc.tensor.matmul(pbg, ones1[:, :64], rsg, start=True, stop=True)
        rsgb = longp.tile([64, NB], FP, tag="rsgb")
        nc.vector.tensor_copy(rsgb, pbg)

        def readout_finish(ro64, gtag):
            gpre = scr.tile([64, NB], FP, tag="sc", name="gpre")
            nc.vector.tensor_mul(gpre, ro64, rsgb)
            sq = scr.tile([64, NB], FP, tag="sc", name="sq")
            nc.scalar.activation(sq, gpre, AF.Square)
            pn = psR.tile([1, NB], FP, tag="row", name="pn")
            nc.tensor.matmul(pn, ones64, sq, start=True, stop=True)
            rn = rows.tile([1, NB], FP, tag="rows", name="rn")
            act_recip(rn, pn, scale=-0.5, clamp=1e-24)
            pb2 = psMM.tile([64, NB], FP, tag="mm", name="pb2")
            nc.tensor.matmul(pb2, ones1[:, :64], rn, start=True, stop=True)
            gg = scr.tile([64, NB], FP, tag="sc", name="gg")
            nc.vector.tensor_mul(gg, gpre, pb2)
            g = longp.tile([64, NB], FP, tag=gtag, name="g")
            nc.scalar.activation(g, gg, AF.Sigmoid)
            return g

        def bilinear(x, y, out_ap):
            pu = psMM.tile([64, NB], FP, tag="mm", name="pu")
            nc.tensor.matmul(pu, bilwT_sb, y, start=True, stop=True)
            p = scr.tile([64, NB], FP, tag="sc", name="bp")
            nc.vector.tensor_mul(p, x, pu)
            pr = psR.tile([1, NB], FP, tag="row", name="pr")
            nc.tensor.matmul(pr, ones64, p, start=True, stop=True)
            nc.scalar.activation(out_ap, pr, AF.Identity, bias=bilb_sb[:, 0:1])

        # g1 branch finishes while the g2 branch still waits on za-AG
        g1 = readout_finish(rog[0:64, :], "g1")
        dm_g = dmlp(g1, "dmg")
        r00 = longp.tile([1, NB], FP, tag="r00")
        r01 = longp.tile([1, NB], FP, tag="r01")
        bilinear(dm_e, dm_g, r00)
        bilinear(dm_ea, dm_g, r01)
        nc.sync.dma_start(io["ret_T"][0:1, :], r00)
        nc.sync.dma_start(io["ret_T"][1:2, :], r01)

        # embaO tiles, then second readout pass
        embaO = embp.tile([128, JT, 65], BF, tag="embO")
        nc.vector.memset(embaO[:, :, 64:65], 1.0)
        for c8 in range(NCORES):
            zc = gchunk.tile([66, NB], FP, tag="gch")
            nc.sync.dma_start(zc[:64, :], zaF_d[c8])
            ptt = psT.tile([128, 1024], BF, tag="pt")
            ptf = ptt.bitcast(FP).rearrange("p (k n) -> p k n", k=4)
            for nt8 in range(NT):
                nc.tensor.transpose(
                    ptf[:, nt8, 0:64],
                    zc[:64, nt8 * 128 : (nt8 + 1) * 128],
                    identf[:64, :64],
                )
            jt0 = c8 * NT
            nc.vector.tensor_relu(embaO[:, jt0 : jt0 + 4, 0:64], ptf[:, :, 0:64])

        pro2 = psA.tile([65, NB], FP, tag="acc")
        for jt in range(JT):
            nc.tensor.matmul(
                pro2, embaO[:, jt, :], gnT[:, jt, :],
                start=(jt == 0), stop=(jt == JT - 1),
            )

        ro2s = scr.tile([65, NB], FP, tag="sc", name="ro2s")
        nc.vector.tensor_copy(ro2s, pro2)
        g2 = readout_finish(ro2s[0:64, :], "g2")
        dm_ga = dmlp(g2, "dmga")
        r10 = longp.tile([1, NB], FP, tag="r10")
        r11 = longp.tile([1, NB], FP, tag="r11")
        bilinear(dm_ea, dm_ga, r10)
        bilinear(dm_e, dm_ga, r11)
        nc.sync.dma_start(io["ret_a_T"][0:1, :], r10)
        nc.sync.dma_start(io["ret_a_T"][1:2, :], r11)


_CACHED = {}


def _get_program():
    if "nc" not in _CACHED:
        nc = bass.Bass(num_devices=NCORES)
        _build(nc)
        _split_waits(nc)
        _CACHED["nc"] = nc
    return _CACHED["nc"]


def run(inputs, **kwargs):
    nc = _get_program()
    w_names = [
        "weight1", "weight2", "att_W", "a_src", "a_dst",
        "mlp_W1", "mlp_b1", "mlp_W2", "mlp_b2", "mlp_W3", "mlp_b3",
        "dmlp_W1", "dmlp_b1", "dmlp_W2", "dmlp_b2", "bil_W", "bil_b",
    ]
    ws = {k: np.ascontiguousarray(np.asarray(inputs[k], dtype=np.float32))
          for k in w_names}
    in_maps = []
    for c in range(NCORES):
        sl = slice(c * NB, (c + 1) * NB)
        m = dict(ws)
        for k in ("feat", "feat_a", "adj", "graph_neigh"):
            m[k] = np.ascontiguousarray(
                np.asarray(inputs[k], dtype=np.float32)[sl])
        in_maps.append(m)

    res = run_bass_kernel_spmd(nc, in_maps, core_ids=list(range(NCORES)), **kwargs)
    outs = res.results
    hiden_emb = np.concatenate([o["hiden_emb_T"].T for o in outs], axis=0)
    h = np.concatenate([o["h"] for o in outs], axis=0)
    ret = np.concatenate([o["ret_T"].T for o in outs], axis=0)
    ret_a = np.concatenate([o["ret_a_T"].T for o in outs], axis=0)
    return (hiden_emb, h, ret, ret_a), res


def kernel(**inputs):
    out, _ = run(inputs)
    return out
